# revision 14
# baseline (speedup 1.0000x reference)
"""Trainium2 Bass kernel for MultiHeadDilatedAttention (v2).

Full inputs in, full output out. Sharding: 8 cores = (batch b in 0..3) x
(segment-position half). Each (b, s) pair is an independent attention problem
(attention runs across segments n at fixed position-in-segment s), so each
core handles b = c//2 and 64 of the 128 s values. No collectives: the output
rows t = s*64 + o for a core's s-range form a contiguous chunk of y[b].

v2 changes vs v1 (321 us):
  - x cast to bf16 on host, loaded via HWDGE in two row-halves so QKV
    matmul chains start ~14 us in instead of waiting ~60 us for a SWDGE
    cast-load of the full fp32 tensor.
  - V^T -> V-natural transposes done by DMA XBAR (dma_start transpose)
    instead of PE transposes + vector/scalar copies. For L<32 heads the
    slots are kept 32-aligned by window-shifted transposes; the garbage
    rows that lands in each slot is neutralized by forcing the matching
    smKQ rows to exactly zero (PSUM memset to -3e10 before softmax plus
    a 1e-30 epsilon on the softmax denominator).
  - Output projection exploits the structural sparsity of the concat:
    output offset o = s*64+o only receives heads with dil_h | o, so rows
    are grouped in 4 offset-classes and each class chain contracts only
    over its contributing heads (2.13x fewer PE cycles than dense).
  - y stored as bf16 (host upcasts), halving write traffic + evictions.
  - attention and out-projection interleaved in 16-s chunks (software
    pipelined one chunk apart) to keep the PE busy and ramped.
  - PSUM->SBUF evictions spread round-robin over vector/scalar/gpsimd.
"""

import numpy as np
import ml_dtypes
from contextlib import ExitStack

import concourse.bass as bass
import concourse.mybir as mybir
import concourse.tile as tile
from concourse import bacc
from concourse.masks import make_identity
from concourse.bass_utils import run_bass_kernel_spmd

F32 = mybir.dt.float32
BF16 = mybir.dt.bfloat16
AX = mybir.AxisListType

B, T, E = 4, 8192, 1024
SEG = 128          # segment size (= #s positions overall)
NB = T // SEG      # 64 segments (attention length before dilation)
NS = 64            # s values per core
ROWS = NB * NS     # 4096 rows per core
HROWS = ROWS // 2  # rows per x half (n in [0,32) / [32,64))
DK = 128
H = 4
DILS = [1, 2, 4, 8]
LS = [NB // d for d in DILS]       # [64, 32, 16, 8]
CONTR = [64, 32, 32, 32]           # att contraction rows (slot height)
G = [2, 4, 4, 4]                   # partition slots used per head
SLOT = [64, 32, 32, 32]            # slot stride
MG = [8, 4, 4, 4]                  # m-groups per KQ psum tile (16 s/chunk)
MOFF = [0, 64, 96, 112]            # mask column offsets, widths LS
VPAD = [0, 0, 16, 24]              # vt column padding for shifted windows
NG = [32, 16, 16, 16]              # vnat [128,128] groups per head
NORM = float(1.0 / np.sqrt(DK))
NEG = -1.0e10
PSNEG = -3.0e10
NECHUNK = E // 128                 # 8
USE_DMA_TRANSPOSE = False          # XBAR transpose races ahead of its sem
SCHUNK = 16                        # s values per phase-D/E chunk
NCHUNK = NS // SCHUNK              # 4

# out-projection classes: (o0, ostep, o-count, heads, s-per-tile)
CLASSES = [
    (1, 2, 32, (0,), 4),           # o odd           -> head 0 only
    (2, 4, 16, (0, 1), 8),         # o = 2 mod 4     -> heads 0,1
    (4, 8, 8, (0, 1, 2), 16),      # o = 4 mod 8     -> heads 0,1,2
    (0, 8, 8, (0, 1, 2, 3), 16),   # o = 0 mod 8     -> all heads
]


def _vnat_loc(h, s):
    """(group, slot) of V_s inside vnat[h]."""
    if h == 0:
        return s // 2, (s % 2) * 64
    if h == 1:
        return s // 4, (s % 4) * 32
    if h == 2:
        return (s // 8) * 2 + (s % 2), ((s % 8) // 2) * 32
    return (s // 16) * 4 + (s % 4), ((s % 16) // 4) * 32


def _slist(h, pi, S0):
    """s values (ci order) handled by partition-slot pi in chunk S0."""
    if h == 0:
        return [S0 + ci * 2 + pi for ci in range(8)]
    if h == 1:
        return [S0 + ci * 4 + pi for ci in range(4)]
    if h == 2:
        return [S0 + 2 * pi + j8 * 8 + j1 for j8 in (0, 1) for j1 in (0, 1)]
    return [S0 + 4 * pi + j for j in range(4)]


def build_program(bias_zero: bool = True, parts=("c", "d", "e")) -> bass.Bass:
    nc = bacc.Bacc("TRN2", target_bir_lowering=False, debug=False)
    xs = nc.dram_tensor("xs", [2, NECHUNK, 128, HROWS], BF16,
                        kind="ExternalInput").ap()
    wqkv = nc.dram_tensor("wqkv", [128, 12 * NECHUNK * 128], BF16,
                          kind="ExternalInput").ap()
    wout = nc.dram_tensor("wout", [128, H * E], BF16, kind="ExternalInput").ap()
    maskd = nc.dram_tensor("masks", [128, 120], F32, kind="ExternalInput").ap()
    biasd = nc.dram_tensor("bias", [128, E], F32, kind="ExternalInput").ap()
    y = nc.dram_tensor("y", [ROWS, E], BF16, kind="ExternalOutput").ap()
    dbg = None
    if "dbg" in parts:
        dbg = {
            "at": nc.dram_tensor("dbg_at", [H, 128, NS * 64], BF16,
                                 kind="ExternalOutput").ap(),
            "vn": nc.dram_tensor("dbg_vn", [H, 128, 32 * 128], BF16,
                                 kind="ExternalOutput").ap(),
            "qk": nc.dram_tensor("dbg_qk", [H, 2, 128, 64 * 64 + 24], BF16,
                                 kind="ExternalOutput").ap(),
        }

    _build_body(nc, xs, wqkv, wout, maskd, biasd, y, bias_zero, parts, dbg)
    nc.finalize()
    return nc


def _build_body(nc, xs, wqkv, wout, maskd, biasd, y, bias_zero, parts=("c", "d", "e"), dbg=None):
    with ExitStack() as ctx:
        tc = ctx.enter_context(tile.TileContext(nc))

        persist = ctx.enter_context(tc.tile_pool(name="persist", bufs=1))
        ident = None
        if not USE_DMA_TRANSPOSE:
            ident = persist.tile([128, 128], BF16, tag="ident")
            make_identity(nc, ident)
        wout_sb = persist.tile([128, H * E], BF16, tag="wout_sb")
        mask_sb = persist.tile([128, 120], F32, tag="mask_sb")
        bias_sb = None
        if not bias_zero:
            bias_sb = persist.tile([128, E], F32, tag="bias_sb")

        # persistent per-head tensors
        qkvpool = ctx.enter_context(tc.tile_pool(name="qkv", bufs=1))
        qkv_sb = {}
        for h in range(H):
            for p in range(3):
                pad = VPAD[h] if p == 2 else 0
                qkv_sb[(h, p)] = qkvpool.tile(
                    [128, LS[h] * NS + pad], BF16,
                    tag=f"qkv{h}{p}", name=f"qkv{h}{p}")
        vnatpool = ctx.enter_context(tc.tile_pool(name="vnat", bufs=1))
        vnat = [vnatpool.tile([128, NG[h] * 128], BF16, tag=f"vnat{h}",
                              name=f"vnat{h}") for h in range(H)]
        atpool = ctx.enter_context(tc.tile_pool(name="atT", bufs=1))
        atT = [atpool.tile([128, NS * 64], BF16, tag=f"atT{h}",
                           name=f"atT{h}") for h in range(H)]

        # ---- phase A: queue DMAs (x on sync queue, weights on scalar) ----
        w_pool = ctx.enter_context(tc.tile_pool(name="w", bufs=1))
        w_sb = w_pool.tile([128, 12 * NECHUNK * 128], BF16, tag="w_sb")
        with ExitStack() as pctx:
            xt_pool = pctx.enter_context(tc.tile_pool(name="xt", bufs=1))
            xt = [xt_pool.tile([128, NECHUNK * HROWS], BF16, tag=f"xt{hf}",
                               name=f"xt{hf}") for hf in range(2)]
            for hf in range(2):
                for ec in range(NECHUNK):
                    nc.sync.dma_start(
                        out=xt[hf][:, ec * HROWS:(ec + 1) * HROWS],
                        in_=xs[hf, ec])
            WCH = NECHUNK * 128
            for h in range(H):
                for p in (2, 0, 1):
                    wi = (h * 3 + p) * WCH
                    nc.scalar.dma_start(out=w_sb[:, wi:wi + WCH],
                                        in_=wqkv[:, wi:wi + WCH])
            nc.scalar.dma_start(out=mask_sb, in_=maskd)
            nc.scalar.dma_start(out=wout_sb, in_=wout)
            if not bias_zero:
                nc.scalar.dma_start(out=bias_sb, in_=biasd)
            # init vt pad columns (read by shifted transpose windows)
            for h in (2, 3):
                L = LS[h]
                nc.gpsimd.memset(qkv_sb[(h, 2)][:, L * NS:], 0.0)

            # ---- phase B: QKV projection, half by half -------------------
            qk_ps = pctx.enter_context(
                tc.tile_pool(name="qk_ps", bufs=6, space="PSUM"))
            ev_engines = [nc.vector.tensor_copy, nc.scalar.copy]
            ev_i = 0
            for hf in range(2):
                xt_r = xt[hf].rearrange("p (e n s) -> p e n s",
                                        e=NECHUNK, s=NS)
                for h in range(H):
                    L, dil = LS[h], DILS[h]
                    lhalf = L // 2
                    ncols = lhalf * NS
                    for p in (2, 0, 1):
                        dst = qkv_sb[(h, p)]
                        for nt in range((ncols + 511) // 512):
                            cw = min(512, ncols - nt * 512)
                            nl = cw // NS
                            l0 = nt * (512 // NS)
                            ps = qk_ps.tile([128, 512], F32)
                            for ec in range(NECHUNK):
                                wi = ((h * 3 + p) * NECHUNK + ec) * 128
                                rhs = xt_r[:, ec,
                                           l0 * dil:(l0 + nl) * dil:dil, :]
                                nc.tensor.matmul(
                                    ps[:, :cw], w_sb[:, wi:wi + 128], rhs,
                                    start=(ec == 0), stop=(ec == NECHUNK - 1))
                            gl0 = hf * lhalf + l0
                            ev = ev_engines[ev_i % 2]
                            ev_i += 1
                            if p == 2:
                                # V^T stored s-major (col = s*L + l)
                                out_ap = dst[:, :L * NS].rearrange(
                                    "p (s l) -> p l s", l=L)[:, gl0:gl0 + nl, :]
                                in_ap = ps[:, :cw].rearrange(
                                    "p (l s) -> p l s", s=NS)
                                ev(out=out_ap, in_=in_ap)
                            else:
                                c0 = gl0 * NS
                                ev(out=dst[:, c0:c0 + cw], in_=ps[:, :cw])

            if "d" not in parts:
                # dummy y write so partial variants have a defined output
                nc.sync.dma_start(out=y[0:128, :],
                                  in_=w_sb[:, 0:E])
        if "c" not in parts or "d" not in parts:
            return
        # ---- phase D+E: attention + out-projection, 16-s chunks ----------
        with ExitStack() as pctx:
            vt_ps = pctx.enter_context(
                tc.tile_pool(name="vt_ps", bufs=2, space="PSUM"))
            kq_ps = pctx.enter_context(
                tc.tile_pool(name="kq_ps", bufs=2, space="PSUM"))
            at_ps = pctx.enter_context(
                tc.tile_pool(name="at_ps", bufs=2, space="PSUM"))
            y_ps = pctx.enter_context(
                tc.tile_pool(name="y_ps", bufs=2, space="PSUM"))
            sm_pool = pctx.enter_context(tc.tile_pool(name="sm", bufs=3))
            small = pctx.enter_context(tc.tile_pool(name="small", bufs=4))
            yo_pool = pctx.enter_context(tc.tile_pool(name="y_sb", bufs=3))
            sc_engines = [nc.vector.tensor_copy, nc.scalar.copy]
            state = {"sc": 0, "ev": 0, "tq": 0, "st": 0}

            def vtranspose(S0):
                # V^T windows -> V natural for this chunk's s values.
                # batched: all of a head's groups transpose into one PSUM
                # tile, one copy evicts them into contiguous vnat cols.
                for h in range(H):
                    L = LS[h]
                    vt = qkv_sb[(h, 2)]
                    if h == 0:
                        g0 = S0 // 2
                        c0s = [g * 2 * L for g in range(g0, g0 + 8)]
                    elif h == 1:
                        g0 = S0 // 4
                        c0s = [g * 4 * L for g in range(g0, g0 + 4)]
                    elif h == 2:
                        g0 = (S0 // 8) * 2
                        c0s = [(S0 + w * 8 + k) * L
                               for w in range(2) for k in range(2)]
                    else:
                        g0 = (S0 // 16) * 4
                        c0s = [(S0 + k) * L for k in range(4)]
                    ng = len(c0s)
                    pt = vt_ps.tile([128, 1024], BF16, tag="vt")
                    for i, c0 in enumerate(c0s):
                        nc.tensor.transpose(pt[:, i * 128:(i + 1) * 128],
                                            vt[:, c0:c0 + 128], ident)
                    cp = sc_engines[state["tq"] % 2]
                    state["tq"] += 1
                    cp(out=vnat[h][:, g0 * 128:(g0 + ng) * 128],
                       in_=pt[:, :ng * 128])

            def kq_softmax(S0):
                # KQ + softmax for all heads; returns live smkq tiles.
                smkqs = {}
                for h in range(H):
                    L, g, sl, mg, cl = LS[h], G[h], SLOT[h], MG[h], CONTR[h]
                    kt_r = qkv_sb[(h, 1)].rearrange("p (l s) -> p l s", s=NS)
                    qt_r = qkv_sb[(h, 0)].rearrange("p (l s) -> p l s", s=NS)
                    m_sl = mask_sb[:, MOFF[h]:MOFF[h] + L]
                    ps_kq = kq_ps.tile([128, mg * L], F32, tag="kq")
                    if cl > L:
                        # neutralize slot garbage rows -> exp == 0 (matmuls
                        # overwrite the real rows below)
                        nc.vector.memset(ps_kq, PSNEG)
                    for ci in range(mg):
                        for pi in range(g):
                            s = _slist(h, pi, S0)[ci]
                            nc.tensor.matmul(
                                ps_kq[pi * sl:pi * sl + L,
                                      ci * L:(ci + 1) * L],
                                kt_r[:, :, s], qt_r[:, :, s],
                                start=True, stop=True,
                                tile_position=(0, pi * sl))
                    numer = sm_pool.tile([128, mg * L], F32, tag="numer")
                    enumer = sm_pool.tile([128, mg * L], BF16, tag="enumer")
                    sums = small.tile([128, mg], F32, tag="sums")
                    recip = small.tile([128, mg], F32, tag="recip")
                    smkq = sm_pool.tile([128, mg * L], BF16, tag="smkq",
                                        bufs=4)
                    mask_bc = bass.AP(tensor=m_sl.tensor, offset=m_sl.offset,
                                      ap=[m_sl.ap[0], [0, mg], m_sl.ap[1]])
                    nc.vector.tensor_add(numer, ps_kq, mask_bc)
                    nc.scalar.activation(
                        enumer, numer,
                        mybir.ActivationFunctionType.Exp, scale=NORM)
                    nc.vector.reduce_sum(
                        sums, enumer.rearrange("p (c l) -> p c l", l=L),
                        axis=AX.X)
                    if cl > L:
                        nc.vector.tensor_scalar_add(sums, sums, 1e-30)
                    nc.vector.reciprocal(recip, sums)
                    rc_bc = bass.AP(tensor=recip.tensor, offset=recip.offset,
                                    ap=[recip.ap[0], [1, mg], [0, L]])
                    nc.vector.tensor_mul(smkq, enumer, rc_bc)
                    smkqs[h] = smkq
                return smkqs

            def att_tile(S0, smkqs, h, pi):
                L, g, sl, mg, cl = LS[h], G[h], SLOT[h], MG[h], CONTR[h]
                smkq = smkqs[h]
                dil = DILS[h]
                slot = pi * sl
                ps_at = at_ps.tile([128, 512], F32, tag="at")
                for ci in range(mg):
                    s = _slist(h, pi, S0)[ci]
                    gi, vslot = _vnat_loc(h, s)
                    assert vslot == slot
                    lhsT = vnat[h][slot:slot + cl,
                                   gi * 128:(gi + 1) * 128]
                    rhs = smkq[slot:slot + cl, ci * L:(ci + 1) * L]
                    nc.tensor.matmul(
                        ps_at[:, ci * L:(ci + 1) * L], lhsT, rhs,
                        start=True, stop=True,
                        tile_position=(slot, 0))
                # scatter into dense atT[h] at cols s*64 + dil*l
                at_r = atT[h].rearrange("p (s o) -> p s o", o=64)
                ps_r = ps_at[:, :mg * L]
                if h == 0:
                    in_ap = ps_r.rearrange("p (c l) -> p c l", l=L)
                    out_ap = at_r[:, S0 + pi:S0 + pi + 15:2, :]
                elif h == 1:
                    in_ap = ps_r.rearrange("p (c l) -> p c l", l=L)
                    out_ap = at_r[:, S0 + pi:S0 + pi + 13:4,
                                  0:L * dil:dil]
                elif h == 2:
                    # ci = j8*2 + j1 -> s = S0 + j8*8 + 2*pi + j1
                    in_ap = ps_r.rearrange(
                        "p (j8 j1 l) -> p j8 j1 l", j8=2, j1=2)
                    out_ap = atT[h].rearrange(
                        "p (s2 s1 o) -> p s2 s1 o", s1=8, o=64)[
                        :, S0 // 8:S0 // 8 + 2,
                        2 * pi:2 * pi + 2, 0:L * dil:dil]
                else:
                    in_ap = ps_r.rearrange("p (c l) -> p c l", l=L)
                    out_ap = at_r[:, S0 + 4 * pi:S0 + 4 * pi + 4,
                                  0:L * dil:dil]
                eng = sc_engines[state["sc"] % 2]
                state["sc"] += 1
                eng(out=out_ap, in_=in_ap)

            def outproj_tile(S0, o0, ostep, no, heads, ns, k, half):
                s0 = S0 + k * ns
                cs = half * 512
                ps_y = y_ps.tile([128, 512], F32, tag="y")
                for idx, h in enumerate(heads):
                    lhsT = atT[h].rearrange(
                        "p (s o) -> p s o", o=64)[
                        :, s0:s0 + ns,
                        o0:o0 + (no - 1) * ostep + 1:ostep]
                    nc.tensor.matmul(
                        ps_y, lhsT,
                        wout_sb[:, h * E + cs:h * E + cs + 512],
                        start=(idx == 0),
                        stop=(idx == len(heads) - 1))
                y_sb = yo_pool.tile([128, 512], BF16)
                if bias_zero:
                    ev = [nc.vector.tensor_copy,
                          nc.scalar.copy][state["ev"] % 2]
                    state["ev"] += 1
                    ev(out=y_sb, in_=ps_y)
                else:
                    nc.vector.tensor_add(
                        y_sb, ps_y, bias_sb[:, cs:cs + 512])
                out_ap = y.rearrange("(s o) e -> s o e", o=64)[
                    s0:s0 + ns,
                    o0:o0 + (no - 1) * ostep + 1:ostep,
                    cs:cs + 512]
                eng = [nc.sync, nc.scalar][state["st"] % 2]
                state["st"] += 1
                eng.dma_start(out=out_ap, in_=y_sb)

            def outproj_args(S0):
                out = []
                for o0, ostep, no, heads, ns in CLASSES:
                    for k in range(SCHUNK // ns):
                        for half in range(2):
                            out.append((S0, o0, ostep, no, heads, ns, k,
                                        half))
                return out

            for ck in range(NCHUNK):
                S0 = ck * SCHUNK
                vtranspose(S0)
                smk = kq_softmax(S0)
                atiles = [(h, pi) for h in range(H) for pi in range(G[h])]
                otiles = outproj_args((ck - 1) * SCHUNK) \
                    if ("e" in parts and ck > 0) else []
                n = max(len(atiles), len(otiles))
                for i in range(n):
                    if i < len(atiles):
                        att_tile(S0, smk, *atiles[i])
                    if i < len(otiles):
                        outproj_tile(*otiles[i])
            if "e" in parts:
                for args in outproj_args((NCHUNK - 1) * SCHUNK):
                    outproj_tile(*args)
            if dbg is not None:
                for h in range(H):
                    nc.sync.dma_start(out=dbg["at"][h], in_=atT[h])
                    nc.sync.dma_start(out=dbg["vn"][h][:, :NG[h] * 128],
                                      in_=vnat[h])
                    for p in range(2):
                        w = LS[h] * NS + (VPAD[h] if p == 1 else 0)
                        nc.sync.dma_start(
                            out=dbg["qk"][h, p][:, :w],
                            in_=qkv_sb[(h, 2 * p)][:, :w])
    nc.finalize()
    return nc


_NC = {}


def _get_program(bias_zero=True):
    if bias_zero not in _NC:
        _NC[bias_zero] = build_program(bias_zero)
    return _NC[bias_zero]


def _host_inputs(Wk, Wq, Wv, W_out, b_out):
    bf = ml_dtypes.bfloat16
    Wstack = np.stack([Wq, Wk, Wv], 1)                     # [H, 3, 128, 1024]
    tmp = Wstack.reshape(H, 3, 128, NECHUNK, 128)          # [h, p, c, ec, r]
    wqkv_sb = np.ascontiguousarray(
        tmp.transpose(4, 0, 1, 3, 2)).reshape(128, -1).astype(bf)
    wout_sb = np.ascontiguousarray(
        W_out.reshape(E, H, 128).transpose(2, 1, 0)).reshape(128, H * E
                                                             ).astype(bf)
    mask_host = np.full((128, 120), NEG, np.float32)
    for h in range(H):
        L, sl = LS[h], SLOT[h]
        for p in range(128):
            n = p % sl
            if n < L:
                mask_host[p, MOFF[h]:MOFF[h] + n + 1] = 0.0
    bias_sb = np.ascontiguousarray(
        np.broadcast_to(np.asarray(b_out, np.float32).reshape(1, E),
                        (128, E)))
    return wqkv_sb, wout_sb, mask_host, bias_sb


def _shard_x(x16, c):
    """x16: bf16 [B, T, E]. Device layout [half, ec, 128, HROWS]."""
    b, half = c // 2, c % 2
    xs = x16[b].reshape(NB, SEG, E)[:, half * NS:(half + 1) * NS, :]
    xs = xs.reshape(2, HROWS, NECHUNK, 128)        # [half, row, ec, e]
    return np.ascontiguousarray(xs.transpose(0, 2, 3, 1))


def prepare(x, Wk, Wq, Wv, W_out, b_out):
    x16 = np.asarray(x, np.float32).astype(ml_dtypes.bfloat16)
    wqkv_sb, wout_sb, mask_host, bias_sb = _host_inputs(
        np.asarray(Wk, np.float32), np.asarray(Wq, np.float32),
        np.asarray(Wv, np.float32), np.asarray(W_out, np.float32),
        np.asarray(b_out, np.float32))
    bias_zero = not np.any(np.asarray(b_out))
    in_maps = []
    for c in range(8):
        in_maps.append({"xs": _shard_x(x16, c), "wqkv": wqkv_sb,
                        "wout": wout_sb, "masks": mask_host,
                        "bias": bias_sb})
    return in_maps, bias_zero


def assemble(res):
    y = np.empty((B, T, E), np.float32)
    for c in range(8):
        b, half = c // 2, c % 2
        y[b, half * ROWS:(half + 1) * ROWS, :] = \
            np.asarray(res.results[c]["y"], dtype=np.float32)
    return y


def kernel(x, Wk, Wq, Wv, W_out, b_out):
    in_maps, bias_zero = prepare(x, Wk, Wq, Wv, W_out, b_out)
    nc = _get_program(bias_zero)
    res = run_bass_kernel_spmd(nc, in_maps, core_ids=list(range(8)))
    return assemble(res)
